# revision 1
# baseline (speedup 1.0000x reference)
"""Trainium2 Bass kernel for BitLinear: y[b,s,o] = sum_d x[b,s,d] * w[o,d].

x: [4, 2048, 4096] f32, weight: [4096, 4096] int32 (values 0..255), y f32.

Strategy:
- Data-parallel over tokens: 8192 tokens -> 8 cores x 1024 tokens.
- Precision: split x = x_hi + x_lo (both bf16; exact to ~2^-18 combined),
  weight values 0..255 are exact in bf16. Stack hi/lo along the contraction
  dim (K = 2*4096 = 8192) and duplicate W^T along K, so a single bf16
  matmul with fp32 PSUM accumulation computes the f32-accurate product.
- Per core: y_shard[1024, 4096] = kxm[8192, 1024]^T @ kxn[8192, 4096].
"""

import sys

for _p in ("/opt/trn_rl_repo", "/root/.axon_site/_ro/trn_rl_repo"):
    if _p not in sys.path:
        sys.path.append(_p)

import numpy as np
import ml_dtypes

N_CORES = 8
TOKENS = 8192  # 4 * 2048
D_IN = 4096
D_OUT = 4096
T_SHARD = TOKENS // N_CORES  # 1024
K2 = 2 * D_IN  # hi/lo stacked

_NC_CACHE = {}


def build_nc():
    """Build (and cache) the Bass program: one bf16 matmul
    [K2, T_SHARD]^T @ [K2, D_OUT] -> [T_SHARD, D_OUT] f32."""
    if "nc" in _NC_CACHE:
        return _NC_CACHE["nc"]

    import concourse.mybir as mybir
    import concourse.tile as tile
    from concourse import bacc
    from concourse.kernels.tile_matmul import matmul_tile_kernel

    nc = bacc.Bacc(None, target_bir_lowering=False)
    with tile.TileContext(nc) as tc:
        with tc.tile_pool(name="dram", bufs=1, space="DRAM") as dram:
            kxm = dram.tile(
                [K2, T_SHARD], mybir.dt.bfloat16,
                kind="ExternalInput", name="kxm", uniquify=False,
            )
            kxn = dram.tile(
                [K2, D_OUT], mybir.dt.bfloat16,
                kind="ExternalInput", name="kxn", uniquify=False,
            )
            mxn = dram.tile(
                [T_SHARD, D_OUT], mybir.dt.float32,
                kind="ExternalOutput", name="mxn", uniquify=False,
            )
            matmul_tile_kernel(tc, kxm[:], kxn[:], mxn[:])
    nc.compile()
    _NC_CACHE["nc"] = nc
    return nc


def prepare_in_maps(x: np.ndarray, weight: np.ndarray):
    """Host-side shard prep: bf16 hi/lo split of x, stacked along K;
    W^T duplicated along K (exact in bf16)."""
    bf16 = ml_dtypes.bfloat16
    x2 = np.ascontiguousarray(x.reshape(TOKENS, D_IN))
    xh = x2.astype(bf16)
    xl = (x2 - xh.astype(np.float32)).astype(bf16)
    # [K2, TOKENS]: hi rows then lo rows
    kxm_full = np.empty((K2, TOKENS), dtype=bf16)
    kxm_full[:D_IN] = xh.T
    kxm_full[D_IN:] = xl.T

    wt = weight.astype(np.float32).astype(bf16).T  # [D_IN, D_OUT], exact
    kxn = np.empty((K2, D_OUT), dtype=bf16)
    kxn[:D_IN] = wt
    kxn[D_IN:] = wt
    kxn = np.ascontiguousarray(kxn)

    in_maps = []
    for c in range(N_CORES):
        kxm_c = np.ascontiguousarray(
            kxm_full[:, c * T_SHARD:(c + 1) * T_SHARD]
        )
        in_maps.append({"kxm": kxm_c, "kxn": kxn})
    return in_maps


def gather_output(results):
    y = np.concatenate(
        [results[c]["mxn"] for c in range(N_CORES)], axis=0
    )
    return y.reshape(4, 2048, D_OUT).astype(np.float32, copy=False)


def kernel(x: np.ndarray, weight: np.ndarray) -> np.ndarray:
    from concourse.bass_utils import run_bass_kernel_spmd

    nc = build_nc()
    in_maps = prepare_in_maps(x, weight)
    res = run_bass_kernel_spmd(nc, in_maps, core_ids=list(range(N_CORES)))
    return gather_output(res.results)


# revision 3
# speedup vs baseline: 63.3485x; 63.3485x over previous
"""Trainium2 Bass kernel for BitLinear: y[b,s,o] = sum_d x[b,s,d] * w[o,d].

x: [4, 2048, 4096] f32, weight: [4096, 4096] int32 (values 0..255), y f32.

Strategy:
- Data-parallel over tokens: 8192 tokens -> 8 cores x 1024 tokens.
- Precision: split x = x_hi + x_lo (both bf16; the pair represents x to
  ~2^-18), weight values 0..255 are exact in bf16. Accumulate both halves
  into the same PSUM bank (fp32) -> near-fp32 result from bf16 matmuls.
- W-stationary formulation: out yt[n, m] = W^T[k, n]^T @ X^T[k, m].
  The hi and lo passes share the SAME stationary W tile, and the two
  512-token moving chunks reuse it too -> 4 consecutive matmuls per
  128-column weight load (weight-load overhead measured ~23 ns/MM when
  reloading every MM; ~1/4 of that here).
- X^T shard (16 MB bf16) is streamed into SBUF during the first output
  group and stays resident; W^T (32 MB, single copy) streams per k-tile.
- Host gathers per-core yt [4096, 1024] f32, transposes, concatenates.
"""

import sys

for _p in ("/opt/trn_rl_repo", "/root/.axon_site/_ro/trn_rl_repo"):
    if _p not in sys.path:
        sys.path.append(_p)

import numpy as np
import ml_dtypes

N_CORES = 8
TOKENS = 8192  # 4 * 2048
D_IN = 4096
D_OUT = 4096
T_SHARD = TOKENS // N_CORES  # 1024
K2 = 2 * D_IN  # hi/lo stacked rows in kxm

_NC_CACHE = {}


def build_nc(repeats: int = 1):
    """Build (and cache) the Bass program.

    repeats > 1 re-emits the compute body (used only for slope-based HW
    timing; identical output)."""
    if repeats in _NC_CACHE:
        return _NC_CACHE[repeats]

    import concourse.mybir as mybir
    import concourse.tile as tile
    from concourse import bacc

    P = 128
    nc = bacc.Bacc(None, target_bir_lowering=False)
    with tile.TileContext(nc) as tc:
        with tc.tile_pool(name="dram", bufs=1, space="DRAM") as dram:
            kxm = dram.tile([K2, T_SHARD], mybir.dt.bfloat16,
                            kind="ExternalInput", name="kxm", uniquify=False)
            kxns = dram.tile([D_IN, D_OUT], mybir.dt.bfloat16,
                             kind="ExternalInput", name="kxns", uniquify=False)
            yt = dram.tile([D_OUT, T_SHARD], mybir.dt.float32,
                           kind="ExternalOutput", name="yt", uniquify=False)
            kxm3 = kxm[:].rearrange("(ko p) m -> p ko m", p=P)  # [128, 64, 1024]
            with tc.tile_pool(name="xpool", bufs=64) as xpool, \
                 tc.tile_pool(name="wpool", bufs=4) as wpool, \
                 tc.tile_pool(name="pspool", bufs=2, space="PSUM") as pspool, \
                 tc.tile_pool(name="evpool", bufs=4) as evpool:
                xtiles = [None] * 64
                NG = D_OUT // 256   # 16 groups of 256 output features
                KT = D_IN // P      # 32 k-tiles
                MC = T_SHARD // 512  # 2 moving chunks of 512 tokens
                first = True
                for _ in range(repeats):
                    for ng in range(NG):
                        banks = {}
                        for nsl in range(2):
                            for mc in range(MC):
                                banks[(nsl, mc)] = pspool.tile(
                                    [P, 512], mybir.dt.float32,
                                    name=f"bank_{nsl}_{mc}",
                                    tag=f"bank_{nsl}_{mc}")
                        for k in range(KT):
                            wt = wpool.tile([P, 256], mybir.dt.bfloat16,
                                            name="wt", tag="wt")
                            nc.sync.dma_start(
                                wt[:], kxns[k * P:(k + 1) * P,
                                            ng * 256:(ng + 1) * 256])
                            if first:  # JIT-load the (hi, lo) xtile pair so
                                # X streaming hides under ng=0 compute
                                for ko in (k, 32 + k):
                                    xt = xpool.tile([P, T_SHARD],
                                                    mybir.dt.bfloat16,
                                                    name="xt", tag="xt")
                                    nc.sync.dma_start(xt[:], kxm3[:, ko])
                                    xtiles[ko] = xt
                            for nsl in range(2):
                                lhsT = wt[:, nsl * P:(nsl + 1) * P]
                                for half in range(2):
                                    for mc in range(MC):
                                        nc.tensor.matmul(
                                            banks[(nsl, mc)][:],
                                            lhsT,
                                            xtiles[half * 32 + k][
                                                :, mc * 512:(mc + 1) * 512],
                                            start=(k == 0 and half == 0),
                                            stop=(k == KT - 1 and half == 1),
                                        )
                        first = False
                        for nsl in range(2):
                            for mc in range(MC):
                                ev = evpool.tile([P, 512], mybir.dt.float32,
                                                 name="ev", tag="ev")
                                nc.vector.tensor_copy(
                                    out=ev[:], in_=banks[(nsl, mc)][:])
                                nc.sync.dma_start(
                                    yt[ng * 256 + nsl * P:
                                       ng * 256 + (nsl + 1) * P,
                                       mc * 512:(mc + 1) * 512],
                                    ev[:])
    nc.compile()
    _NC_CACHE[repeats] = nc
    return nc


def prepare_in_maps(x: np.ndarray, weight: np.ndarray):
    """Host-side shard prep: bf16 hi/lo split of x stacked along K;
    W^T as a single bf16 copy (values 0..255 are exact)."""
    bf16 = ml_dtypes.bfloat16
    x2 = np.ascontiguousarray(np.asarray(x).reshape(TOKENS, D_IN))
    xh = x2.astype(bf16)
    xl = (x2 - xh.astype(np.float32)).astype(bf16)
    kxm_full = np.empty((K2, TOKENS), dtype=bf16)
    kxm_full[:D_IN] = xh.T
    kxm_full[D_IN:] = xl.T

    wt = np.ascontiguousarray(
        np.asarray(weight).astype(np.float32).astype(bf16).T)  # [D_IN, D_OUT]

    in_maps = []
    for c in range(N_CORES):
        kxm_c = np.ascontiguousarray(
            kxm_full[:, c * T_SHARD:(c + 1) * T_SHARD])
        in_maps.append({"kxm": kxm_c, "kxns": wt})
    return in_maps


def gather_output(results):
    y = np.concatenate(
        [np.ascontiguousarray(results[c]["yt"].T) for c in range(N_CORES)],
        axis=0)
    return y.reshape(4, 2048, D_OUT).astype(np.float32, copy=False)


def kernel(x: np.ndarray, weight: np.ndarray) -> np.ndarray:
    from concourse.bass_utils import run_bass_kernel_spmd

    nc = build_nc()
    in_maps = prepare_in_maps(x, weight)
    res = run_bass_kernel_spmd(nc, in_maps, core_ids=list(range(N_CORES)))
    return gather_output(res.results)
